# revision 18
# baseline (speedup 1.0000x reference)
"""Fused multi-head attention (B=4, S=2048, D=1024, H=16, Dh=64, RoPE) on 8 NeuronCores.

Sharding: core = (batch b, head-group g) with b = core//2, g = core%2.
Each core computes its batch's 8 heads end-to-end (qkv proj, RoPE, attention,
out-proj partial with Wout row-slice); host sums the two partials per batch.

On-device layout is "transposed" (features on partitions, sequence on the free
dim) so no on-device transposes are needed:
  A: qkT/kT = wqk.T @ xT   (f on partitions)  +  v = xT.T @ wv (natural [s, f])
     RoPE applied by DVE straight out of PSUM (sign folded into the sin table).
  B: simT[j,i] = krT.T @ qrT  per head (K=64), exp on ACT with fused 1/8 scale
     (no max subtraction: |sim| is O(6) for these inputs, exp is safe in fp32).
  C: outT_aug = v_aug.T @ expT  with a ones column in v_aug producing the
     softmax denominator for free (M=65).
  normalize: DVE reciprocal + GPSIMD partition broadcast + DVE multiply.
  D: finalT = wout.T @ outT.
All matmuls run in float32r (full PE rate, ~1e-4 relative rounding).
"""
import sys

for p in ("/opt/trn_rl_repo",):
    if p not in sys.path:
        sys.path.insert(0, p)

import numpy as np

import concourse.bacc as bacc
import concourse.bass as bass
import concourse.tile as tile
from concourse import mybir
from concourse.bass_utils import run_bass_kernel_spmd

P = 128
S = 2048
D = 1024
NH = 8            # heads per core
DH = 64
SB = 512          # matmul free-dim block
NSB = S // SB     # 4 s-blocks
KD = D // P       # 8 contraction tiles over d
ST = S // P       # 16 s partition-tiles (keys)
FV = NH * DH      # 512 features for this head group
N_CORES = 8
SCALE = DH ** -0.5

f32 = mybir.dt.float32
f32r = mybir.dt.float32r
bf16 = mybir.dt.float16  # fp16: 10-bit mantissa, values are O(10) so range is safe


def build_program():
    nc = bacc.Bacc("TRN2", target_bir_lowering=False, debug=False,
                   enable_asserts=False, num_devices=N_CORES)

    xT = nc.dram_tensor("xT", [D, S], f32r, kind="ExternalInput").ap()
    wqk = nc.dram_tensor("wqk", [D, 2 * FV], f32r, kind="ExternalInput").ap()
    wv = nc.dram_tensor("wv", [D, FV], f32r, kind="ExternalInput").ap()
    wout = nc.dram_tensor("wout", [FV, D], f32r, kind="ExternalInput").ap()
    cosb = nc.dram_tensor("cosb", [P, S], f32, kind="ExternalInput").ap()
    sinb = nc.dram_tensor("sinb", [P, S], f32, kind="ExternalInput").ap()
    outT = nc.dram_tensor("outT", [D, S], f32, kind="ExternalOutput").ap()

    with tile.TileContext(nc) as tc:
        with tc.tile_pool(name="persist", bufs=1) as pp, \
             tc.tile_pool(name="dram", bufs=1, space="DRAM") as dp, \
             tc.tile_pool(name="psum", bufs=1, space="PSUM") as psp:
            v_sb = [pp.tile([P, NH * (DH + 1)], f32r, tag=f"v{i}", name=f"v{i}") for i in range(ST)]
            outT_sb = [pp.tile([P, S], f32r, tag=f"ot{t}", name=f"ot{t}") for t in range(NSB)]
            # cos/sin partial products round-trip through DRAM in fp16 (one
            # tile per head pair); the rotate-half partition swap happens in
            # the read-back addressing.
            qc_d = [dp.tile([P, S], bf16, tag=f"qc_d{t}", name=f"qc_d{t}") for t in range(NSB)]
            kc_d = [dp.tile([P, S], bf16, tag=f"kc_d{t}", name=f"kc_d{t}") for t in range(NSB)]
            qs_d = [dp.tile([P, S], bf16, tag=f"qs_d{t}", name=f"qs_d{t}") for t in range(NSB)]
            ks_d = [dp.tile([P, S], bf16, tag=f"ks_d{t}", name=f"ks_d{t}") for t in range(NSB)]

            # ones columns of v_aug
            ones8 = pp.tile([P, NH], f32, tag="ones8", name="ones8")
            nc.vector.memset(ones8[:], 1.0)
            for i in range(ST):
                ones_dst = v_sb[i].rearrange("p (h e) -> p h e", h=NH)[:, :, DH]
                nc.vector.tensor_copy(ones_dst, ones8[:])

            with tc.tile_pool(name="qkph", bufs=1) as qkph, \
                 tc.tile_pool(name="qks", bufs=1) as qks, \
                 tc.tile_pool(name="expp", bufs=3) as expp, \
                 tc.tile_pool(name="nump", bufs=3) as nump, \
                 tc.tile_pool(name="bcp", bufs=3) as bcp, \
                 tc.tile_pool(name="rrp", bufs=2) as rrp, \
                 tc.tile_pool(name="doutp", bufs=2) as doutp:

                wv_sb = [qkph.tile([P, FV], f32r, tag=f"wv{k}", name=f"wv{k}")
                         for k in range(KD)]
                for k in range(KD):
                    nc.sync.dma_start(wv_sb[k][:], wv[P * k:P * (k + 1), :])

                def emit_a_group(pairs, with_v):
                    """Stream xT once; project v (optionally) and the q/k
                    feature tiles for `pairs`, write fp16 RoPE partials."""
                    wsl = {}
                    for pi, t in enumerate(pairs):
                        for qk in range(2):
                            col0 = FV * qk + P * t
                            tiles = [qkph.tile([P, P], f32r, tag=f"w{pi}_{qk}_{k}",
                                               name="wsl") for k in range(KD)]
                            for k in range(KD):
                                nc.sync.dma_start(
                                    tiles[k][:], wqk[P * k:P * (k + 1), col0:col0 + P])
                            wsl[(t, qk)] = tiles
                    for nb in range(NSB):
                        sl = slice(nb * SB, (nb + 1) * SB)
                        xts = [qkph.tile([P, SB], f32r, tag=f"xt{k}", bufs=2,
                                         name=f"xt{k}") for k in range(KD)]
                        for k in range(KD):
                            nc.sync.dma_start(xts[k][:], xT[P * k:P * (k + 1), sl])
                        cos_sb = qkph.tile([P, SB], f32, tag="cos", bufs=2, name="cos_sb")
                        sin_sb = qkph.tile([P, SB], f32, tag="sin", bufs=2, name="sin_sb")
                        nc.sync.dma_start(cos_sb[:], cosb[:, sl])
                        nc.sync.dma_start(sin_sb[:], sinb[:, sl])

                        for t in pairs:
                            for qk in range(2):
                                ps = psp.tile([P, SB], f32, tag="psA", bufs=2, name="ps")
                                for k in range(KD):
                                    nc.tensor.matmul(ps[:], wsl[(t, qk)][k][:],
                                                     xts[k][:],
                                                     start=(k == 0), stop=(k == KD - 1))
                                pc = qkph.tile([P, SB], bf16, tag="pc", bufs=3, name="pc")
                                psn = qkph.tile([P, SB], bf16, tag="psn", bufs=3, name="psn")
                                nc.vector.tensor_mul(pc[:], ps[:], cos_sb[:])
                                nc.vector.tensor_mul(psn[:], ps[:], sin_sb[:])
                                ctgt = (qc_d if qk == 0 else kc_d)[t]
                                stgt = (qs_d if qk == 0 else ks_d)[t]
                                nc.sync.dma_start(ctgt[:, sl], pc[:])
                                nc.sync.dma_start(stgt[:, sl], psn[:])

                        if with_v:
                            for st in range(NSB):
                                s_idx = nb * NSB + st
                                psv = psp.tile([P, FV], f32, tag="psA", bufs=2, name="psv")
                                for k in range(KD):
                                    nc.tensor.matmul(psv[:], xts[k][:, P * st:P * (st + 1)],
                                                     wv_sb[k][:],
                                                     start=(k == 0), stop=(k == KD - 1))
                                for h in range(NH):
                                    nc.vector.tensor_copy(
                                        v_sb[s_idx][:, (DH + 1) * h:(DH + 1) * h + DH],
                                        psv[:, DH * h:DH * (h + 1)])

                def load_roped(t, c_d, s_d):
                    """Load fp16 cos/sin partials of pair t (sin with the
                    rotate-half partition swap) and add to f32r, per s-block
                    so attention can start before the last block lands."""
                    ct = qks.tile([P, S], bf16, tag="ct", bufs=1, name="ct")
                    sw = qks.tile([P, S], bf16, tag="sw", bufs=1, name="sw")
                    r = qks.tile([P, S], f32r, tag="ropr", bufs=3, name="ropr")
                    for nb in range(NSB):
                        sl = slice(nb * SB, (nb + 1) * SB)
                        nc.sync.dma_start(ct[:, sl], c_d[t][:, sl])
                        for blk in range(4):
                            a = 32 * blk
                            src = 32 * (blk ^ 1)
                            nc.sync.dma_start(sw[a:a + 32, sl], s_d[t][src:src + 32, sl])
                        nc.vector.tensor_add(r[:, sl], ct[:, sl], sw[:, sl])
                    return r

                def emit_bcd_pair(t):
                    qs = load_roped(t, qc_d, qs_d)
                    ks = load_roped(t, kc_d, ks_d)
                    for hh in range(2):
                        h = 2 * t + hh
                        off = DH * hh
                        for ip in range(2):   # i-block pairs
                            augs = [psp.tile([DH + 1, SB], f32, tag=f"aug{ii}",
                                             bufs=1, name=f"aug{ii}")
                                    for ii in range(2)]
                            # software pipeline: emit C(j-2) after B(j)+exp(j)
                            ets = {}

                            def emit_b(j):
                                sim = psp.tile([P, 2 * SB], f32, tag="sim",
                                               bufs=2, name="sim")
                                for ii in range(2):
                                    isl = slice((2 * ip + ii) * SB, (2 * ip + ii + 1) * SB)
                                    nc.tensor.matmul(sim[:, SB * ii:SB * (ii + 1)],
                                                     ks[off:off + DH, P * j:P * (j + 1)],
                                                     qs[off:off + DH, isl],
                                                     start=True, stop=True)
                                et = expp.tile([P, 2 * SB], f32r, tag="exp", name="et")
                                nc.scalar.activation(et[:], sim[:],
                                                     mybir.ActivationFunctionType.Exp,
                                                     scale=SCALE)
                                ets[j] = et

                            def emit_c(j):
                                et = ets.pop(j)
                                for ii in range(2):
                                    nc.tensor.matmul(augs[ii][:],
                                                     v_sb[j][:, (DH + 1) * h:(DH + 1) * h + DH + 1],
                                                     et[:, SB * ii:SB * (ii + 1)],
                                                     start=(j == 0), stop=(j == ST - 1))

                            DEPTH = 2
                            for j in range(ST):
                                emit_b(j)
                                if j >= DEPTH:
                                    emit_c(j - DEPTH)
                            for j in range(ST - DEPTH, ST):
                                emit_c(j)

                            for ii in range(2):
                                i_blk = 2 * ip + ii
                                isl = slice(i_blk * SB, (i_blk + 1) * SB)
                                num = nump.tile([DH, SB], f32, tag="num", name="num")
                                nc.vector.tensor_copy(num[:], augs[ii][0:DH, :])
                                rrow = rrp.tile([1, SB], f32, tag="rrow", name="rrow")
                                nc.vector.reciprocal(rrow[0:1, :], augs[ii][DH:DH + 1, :])
                                bc = bcp.tile([DH, SB], f32, tag="bc", name="bc")
                                nc.gpsimd.partition_broadcast(bc[:], rrow[0:1, :])
                                nc.vector.tensor_mul(outT_sb[t][off:off + DH, isl],
                                                     num[:], bc[:])

                emit_a_group((0,), with_v=True)
                emit_bcd_pair(0)
                emit_a_group((1,), with_v=False)
                emit_bcd_pair(1)
                emit_a_group((2, 3), with_v=False)
                emit_bcd_pair(2)
                emit_bcd_pair(3)

                # ---------------- Phase D: final projection ----------------
                # wout reuses the wv slots (same shape, wv is dead after the
                # v-sweep): tile (k, half) = wout[128k:+128, 512*half:+512]
                wout_sb = [qkph.tile([P, FV], f32r, tag=f"wv{2 * k + half}", name="wo")
                           for k in range(FV // P) for half in range(2)]
                for k in range(FV // P):
                    for half in range(2):
                        nc.sync.dma_start(wout_sb[2 * k + half][:],
                                          wout[P * k:P * (k + 1), FV * half:FV * (half + 1)])
                for mi in range(D // P):
                    for ib in range(NSB):
                        isl = slice(ib * SB, (ib + 1) * SB)
                        pd = psp.tile([P, SB], f32, tag="psA", bufs=2, name="pd")
                        for k in range(FV // P):
                            wt = wout_sb[2 * k + mi // 4]
                            nc.tensor.matmul(pd[:], wt[:, P * (mi % 4):P * (mi % 4 + 1)],
                                             outT_sb[k][:, isl],
                                             start=(k == 0), stop=(k == FV // P - 1))
                        ot = doutp.tile([P, SB], f32, tag="dout", name="dout")
                        nc.vector.tensor_copy(ot[:], pd[:])
                        nc.sync.dma_start(outT[P * mi:P * (mi + 1), isl], ot[:])

    nc.compile()
    return nc


_PROG = None


def _get_prog():
    global _PROG
    if _PROG is None:
        _PROG = build_program()
    return _PROG


def make_in_maps(x, Wqkv, Wout):
    B = x.shape[0]
    HEADS = 16
    BASE = 10000.0
    # RoPE tables, sign folded into sin, 32-row frequency pattern tiled to 128
    f = np.arange(32, dtype=np.float64)
    invfreq = BASE ** (-2.0 * f / DH)                      # [32]
    tpos = np.arange(S, dtype=np.float64)
    ang = np.outer(invfreq, tpos)                          # [32, S]
    cos32 = np.cos(ang)
    sin32 = np.sin(ang)
    cosb = np.tile(cos32, (4, 1)).astype(np.float32)       # [128, S]
    # sign indexed by SOURCE row r: the swap moves row r to row swap(r), which
    # needs -sin when swap(r)%64 < 32, i.e. when r%64 >= 32
    sgn = np.repeat(np.array([1.0, -1.0, 1.0, -1.0]), 32)[:, None]
    sinb = (np.tile(sin32, (4, 1)) * sgn).astype(np.float32)

    in_maps = []
    for c in range(N_CORES):
        b, g = divmod(c, 2)
        xTc = np.ascontiguousarray(x[b].T)                 # [D, S]
        wqk_c = np.ascontiguousarray(
            np.concatenate([Wqkv[:, 512 * g:512 * g + 512],
                            Wqkv[:, 1024 + 512 * g:1024 + 512 * g + 512]], axis=1))
        wv_c = np.ascontiguousarray(Wqkv[:, 2048 + 512 * g:2048 + 512 * g + 512])
        wout_c = np.ascontiguousarray(Wout[512 * g:512 * g + 512, :])
        in_maps.append({"xT": xTc, "wqk": wqk_c, "wv": wv_c, "wout": wout_c,
                        "cosb": cosb, "sinb": sinb})
    return in_maps


def gather_output(results, B=4):
    outs = []
    for b in range(B):
        acc = results[2 * b]["outT"].astype(np.float32) + results[2 * b + 1]["outT"]
        outs.append(acc.T)
    return np.stack(outs, axis=0)


def kernel(x, Wqkv, Wout):
    x = np.asarray(x, dtype=np.float32)
    Wqkv = np.asarray(Wqkv, dtype=np.float32)
    Wout = np.asarray(Wout, dtype=np.float32)
    nc = _get_prog()
    in_maps = make_in_maps(x, Wqkv, Wout)
    res = run_bass_kernel_spmd(nc, in_maps, core_ids=list(range(N_CORES)))
    return gather_output(res.results, B=x.shape[0])


if __name__ == "__main__":
    rng = np.random.default_rng(0)
    x = rng.standard_normal((4, S, D)).astype(np.float32)
    Wqkv = (rng.standard_normal((D, 3 * D)) * D ** -0.5).astype(np.float32)
    Wout = (rng.standard_normal((D, D)) * D ** -0.5).astype(np.float32)
    out = kernel(x, Wqkv, Wout)
    print("kernel ran, out shape:", out.shape, "finite:", np.isfinite(out).all())


# revision 19
# speedup vs baseline: 1.0287x; 1.0287x over previous
"""Fused multi-head attention (B=4, S=2048, D=1024, H=16, Dh=64, RoPE) on 8 NeuronCores.

Sharding: core = (batch b, head-group g) with b = core//2, g = core%2.
Each core computes its batch's 8 heads end-to-end (qkv proj, RoPE, attention,
out-proj partial with Wout row-slice); host sums the two partials per batch.

On-device layout is "transposed" (features on partitions, sequence on the free
dim) so no on-device transposes are needed:
  A: qkT/kT = wqk.T @ xT   (f on partitions)  +  v = xT.T @ wv (natural [s, f])
     RoPE applied by DVE straight out of PSUM (sign folded into the sin table).
  B: simT[j,i] = krT.T @ qrT  per head (K=64), exp on ACT with fused 1/8 scale
     (no max subtraction: |sim| is O(6) for these inputs, exp is safe in fp32).
  C: outT_aug = v_aug.T @ expT  with a ones column in v_aug producing the
     softmax denominator for free (M=65).
  normalize: DVE reciprocal + GPSIMD partition broadcast + DVE multiply.
  D: finalT = wout.T @ outT.
All matmuls run in float32r (full PE rate, ~1e-4 relative rounding).
"""
import sys

for p in ("/opt/trn_rl_repo",):
    if p not in sys.path:
        sys.path.insert(0, p)

import numpy as np

import concourse.bacc as bacc
import concourse.bass as bass
import concourse.tile as tile
from concourse import mybir
from concourse.bass_utils import run_bass_kernel_spmd

P = 128
S = 2048
D = 1024
NH = 8            # heads per core
DH = 64
SB = 512          # matmul free-dim block
NSB = S // SB     # 4 s-blocks
KD = D // P       # 8 contraction tiles over d
ST = S // P       # 16 s partition-tiles (keys)
FV = NH * DH      # 512 features for this head group
N_CORES = 8
SCALE = DH ** -0.5

f32 = mybir.dt.float32
f32r = mybir.dt.float32r
bf16 = mybir.dt.float16  # fp16: 10-bit mantissa, values are O(10) so range is safe


def build_program():
    nc = bacc.Bacc("TRN2", target_bir_lowering=False, debug=False,
                   enable_asserts=False, num_devices=N_CORES)

    xT = nc.dram_tensor("xT", [D, S], f32r, kind="ExternalInput").ap()
    wqk = nc.dram_tensor("wqk", [D, 2 * FV], f32r, kind="ExternalInput").ap()
    wv = nc.dram_tensor("wv", [D, FV], f32r, kind="ExternalInput").ap()
    wout = nc.dram_tensor("wout", [FV, D], f32r, kind="ExternalInput").ap()
    cosb = nc.dram_tensor("cosb", [P, S], f32, kind="ExternalInput").ap()
    sinb = nc.dram_tensor("sinb", [P, S], f32, kind="ExternalInput").ap()
    outT = nc.dram_tensor("outT", [D, S], f32, kind="ExternalOutput").ap()

    with tile.TileContext(nc) as tc:
        with tc.tile_pool(name="persist", bufs=1) as pp, \
             tc.tile_pool(name="dram", bufs=1, space="DRAM") as dp, \
             tc.tile_pool(name="psum", bufs=1, space="PSUM") as psp:
            v_sb = [pp.tile([P, NH * (DH + 1)], f32r, tag=f"v{i}", name=f"v{i}") for i in range(ST)]
            outT_sb = [pp.tile([P, S], f32r, tag=f"ot{t}", name=f"ot{t}") for t in range(NSB)]
            # cos/sin partial products round-trip through DRAM in fp16 (one
            # tile per head pair); the rotate-half partition swap happens in
            # the read-back addressing.
            qc_d = [dp.tile([P, S], bf16, tag=f"qc_d{t}", name=f"qc_d{t}") for t in range(NSB)]
            kc_d = [dp.tile([P, S], bf16, tag=f"kc_d{t}", name=f"kc_d{t}") for t in range(NSB)]
            qs_d = [dp.tile([P, S], bf16, tag=f"qs_d{t}", name=f"qs_d{t}") for t in range(NSB)]
            ks_d = [dp.tile([P, S], bf16, tag=f"ks_d{t}", name=f"ks_d{t}") for t in range(NSB)]

            # ones columns of v_aug
            ones8 = pp.tile([P, NH], f32, tag="ones8", name="ones8")
            nc.vector.memset(ones8[:], 1.0)
            for i in range(ST):
                ones_dst = v_sb[i].rearrange("p (h e) -> p h e", h=NH)[:, :, DH]
                nc.vector.tensor_copy(ones_dst, ones8[:])

            with tc.tile_pool(name="qkph", bufs=1) as qkph, \
                 tc.tile_pool(name="qks", bufs=1) as qks, \
                 tc.tile_pool(name="expp", bufs=3) as expp, \
                 tc.tile_pool(name="nump", bufs=3) as nump, \
                 tc.tile_pool(name="bcp", bufs=3) as bcp, \
                 tc.tile_pool(name="rrp", bufs=2) as rrp, \
                 tc.tile_pool(name="doutp", bufs=2) as doutp:

                wv_sb = [qkph.tile([P, FV], f32r, tag=f"wv{k}", name=f"wv{k}")
                         for k in range(KD)]
                for k in range(KD):
                    nc.sync.dma_start(wv_sb[k][:], wv[P * k:P * (k + 1), :])

                def emit_a_group(pairs, with_v):
                    """Stream xT once; project v (optionally) and the q/k
                    feature tiles for `pairs`, write fp16 RoPE partials."""
                    wsl = {}
                    for pi, t in enumerate(pairs):
                        for qk in range(2):
                            col0 = FV * qk + P * t
                            tiles = [qkph.tile([P, P], f32r, tag=f"w{pi}_{qk}_{k}",
                                               name="wsl") for k in range(KD)]
                            for k in range(KD):
                                nc.sync.dma_start(
                                    tiles[k][:], wqk[P * k:P * (k + 1), col0:col0 + P])
                            wsl[(t, qk)] = tiles
                    for nb in range(NSB):
                        sl = slice(nb * SB, (nb + 1) * SB)
                        xts = [qkph.tile([P, SB], f32r, tag=f"xt{k}", bufs=2,
                                         name=f"xt{k}") for k in range(KD)]
                        for k in range(KD):
                            nc.sync.dma_start(xts[k][:], xT[P * k:P * (k + 1), sl])
                        cos_sb = qkph.tile([P, SB], f32, tag="cos", bufs=2, name="cos_sb")
                        sin_sb = qkph.tile([P, SB], f32, tag="sin", bufs=2, name="sin_sb")
                        nc.sync.dma_start(cos_sb[:], cosb[:, sl])
                        nc.sync.dma_start(sin_sb[:], sinb[:, sl])

                        for t in pairs:
                            for qk in range(2):
                                ps = psp.tile([P, SB], f32, tag="psA", bufs=2, name="ps")
                                for k in range(KD):
                                    nc.tensor.matmul(ps[:], wsl[(t, qk)][k][:],
                                                     xts[k][:],
                                                     start=(k == 0), stop=(k == KD - 1))
                                pc = qkph.tile([P, SB], bf16, tag="pc", bufs=3, name="pc")
                                psn = qkph.tile([P, SB], bf16, tag="psn", bufs=3, name="psn")
                                nc.vector.tensor_mul(pc[:], ps[:], cos_sb[:])
                                nc.vector.tensor_mul(psn[:], ps[:], sin_sb[:])
                                ctgt = (qc_d if qk == 0 else kc_d)[t]
                                stgt = (qs_d if qk == 0 else ks_d)[t]
                                nc.sync.dma_start(ctgt[:, sl], pc[:])
                                nc.sync.dma_start(stgt[:, sl], psn[:])

                        if with_v:
                            for st in range(NSB):
                                s_idx = nb * NSB + st
                                psv = psp.tile([P, FV], f32, tag="psA", bufs=2, name="psv")
                                for k in range(KD):
                                    nc.tensor.matmul(psv[:], xts[k][:, P * st:P * (st + 1)],
                                                     wv_sb[k][:],
                                                     start=(k == 0), stop=(k == KD - 1))
                                for h in range(NH):
                                    nc.vector.tensor_copy(
                                        v_sb[s_idx][:, (DH + 1) * h:(DH + 1) * h + DH],
                                        psv[:, DH * h:DH * (h + 1)])

                def load_roped(t, c_d, s_d):
                    """Load fp16 cos/sin partials of pair t (sin with the
                    rotate-half partition swap) and add to f32r, per s-block
                    so attention can start before the last block lands."""
                    ct = qks.tile([P, S], bf16, tag="ct", bufs=1, name="ct")
                    sw = qks.tile([P, S], bf16, tag="sw", bufs=1, name="sw")
                    r = qks.tile([P, S], f32r, tag="ropr", bufs=3, name="ropr")
                    for nb in range(NSB):
                        sl = slice(nb * SB, (nb + 1) * SB)
                        nc.sync.dma_start(ct[:, sl], c_d[t][:, sl])
                        for blk in range(4):
                            a = 32 * blk
                            src = 32 * (blk ^ 1)
                            nc.sync.dma_start(sw[a:a + 32, sl], s_d[t][src:src + 32, sl])
                        nc.vector.tensor_add(r[:, sl], ct[:, sl], sw[:, sl])
                    return r

                def emit_bcd_pair(t):
                    qs = load_roped(t, qc_d, qs_d)
                    ks = load_roped(t, kc_d, ks_d)
                    for hh in range(2):
                        h = 2 * t + hh
                        off = DH * hh
                        for ip in range(2):   # i-block pairs
                            augs = [psp.tile([DH + 1, SB], f32, tag=f"aug{ii}",
                                             bufs=1, name=f"aug{ii}")
                                    for ii in range(2)]
                            # software pipeline: emit C(j-2) after B(j)+exp(j)
                            ets = {}

                            def emit_b(j):
                                sim = psp.tile([P, 2 * SB], f32, tag="sim",
                                               bufs=2, name="sim")
                                for ii in range(2):
                                    isl = slice((2 * ip + ii) * SB, (2 * ip + ii + 1) * SB)
                                    nc.tensor.matmul(sim[:, SB * ii:SB * (ii + 1)],
                                                     ks[off:off + DH, P * j:P * (j + 1)],
                                                     qs[off:off + DH, isl],
                                                     start=True, stop=True)
                                et = expp.tile([P, 2 * SB], f32r, tag="exp", name="et")
                                nc.scalar.activation(et[:], sim[:],
                                                     mybir.ActivationFunctionType.Exp,
                                                     scale=SCALE)
                                ets[j] = et

                            def emit_c(j):
                                et = ets.pop(j)
                                for ii in range(2):
                                    nc.tensor.matmul(augs[ii][:],
                                                     v_sb[j][:, (DH + 1) * h:(DH + 1) * h + DH + 1],
                                                     et[:, SB * ii:SB * (ii + 1)],
                                                     start=(j == 0), stop=(j == ST - 1))

                            DEPTH = 2
                            for j in range(ST):
                                emit_b(j)
                                if j >= DEPTH:
                                    emit_c(j - DEPTH)
                            for j in range(ST - DEPTH, ST):
                                emit_c(j)

                            for ii in range(2):
                                i_blk = 2 * ip + ii
                                isl = slice(i_blk * SB, (i_blk + 1) * SB)
                                num = nump.tile([DH, SB], f32, tag="num", name="num")
                                nc.vector.tensor_copy(num[:], augs[ii][0:DH, :])
                                rrow = rrp.tile([1, SB], f32, tag="rrow", name="rrow")
                                nc.vector.reciprocal(rrow[0:1, :], augs[ii][DH:DH + 1, :])
                                bc = bcp.tile([DH, SB], f32, tag="bc", name="bc")
                                nc.gpsimd.partition_broadcast(bc[:], rrow[0:1, :])
                                nc.vector.tensor_mul(outT_sb[t][off:off + DH, isl],
                                                     num[:], bc[:])

                emit_a_group((0, 1), with_v=True)
                emit_bcd_pair(0)
                emit_a_group((2, 3), with_v=False)
                emit_bcd_pair(1)
                emit_bcd_pair(2)
                emit_bcd_pair(3)

                # ---------------- Phase D: final projection ----------------
                # wout reuses the wv slots (same shape, wv is dead after the
                # v-sweep): tile (k, half) = wout[128k:+128, 512*half:+512]
                wout_sb = [qkph.tile([P, FV], f32r, tag=f"wv{2 * k + half}", name="wo")
                           for k in range(FV // P) for half in range(2)]
                for k in range(FV // P):
                    for half in range(2):
                        nc.sync.dma_start(wout_sb[2 * k + half][:],
                                          wout[P * k:P * (k + 1), FV * half:FV * (half + 1)])
                for mi in range(D // P):
                    for ib in range(NSB):
                        isl = slice(ib * SB, (ib + 1) * SB)
                        pd = psp.tile([P, SB], f32, tag="psA", bufs=2, name="pd")
                        for k in range(FV // P):
                            wt = wout_sb[2 * k + mi // 4]
                            nc.tensor.matmul(pd[:], wt[:, P * (mi % 4):P * (mi % 4 + 1)],
                                             outT_sb[k][:, isl],
                                             start=(k == 0), stop=(k == FV // P - 1))
                        ot = doutp.tile([P, SB], f32, tag="dout", name="dout")
                        nc.vector.tensor_copy(ot[:], pd[:])
                        nc.sync.dma_start(outT[P * mi:P * (mi + 1), isl], ot[:])

    nc.compile()
    return nc


_PROG = None


def _get_prog():
    global _PROG
    if _PROG is None:
        _PROG = build_program()
    return _PROG


def make_in_maps(x, Wqkv, Wout):
    B = x.shape[0]
    HEADS = 16
    BASE = 10000.0
    # RoPE tables, sign folded into sin, 32-row frequency pattern tiled to 128
    f = np.arange(32, dtype=np.float64)
    invfreq = BASE ** (-2.0 * f / DH)                      # [32]
    tpos = np.arange(S, dtype=np.float64)
    ang = np.outer(invfreq, tpos)                          # [32, S]
    cos32 = np.cos(ang)
    sin32 = np.sin(ang)
    cosb = np.tile(cos32, (4, 1)).astype(np.float32)       # [128, S]
    # sign indexed by SOURCE row r: the swap moves row r to row swap(r), which
    # needs -sin when swap(r)%64 < 32, i.e. when r%64 >= 32
    sgn = np.repeat(np.array([1.0, -1.0, 1.0, -1.0]), 32)[:, None]
    sinb = (np.tile(sin32, (4, 1)) * sgn).astype(np.float32)

    in_maps = []
    for c in range(N_CORES):
        b, g = divmod(c, 2)
        xTc = np.ascontiguousarray(x[b].T)                 # [D, S]
        wqk_c = np.ascontiguousarray(
            np.concatenate([Wqkv[:, 512 * g:512 * g + 512],
                            Wqkv[:, 1024 + 512 * g:1024 + 512 * g + 512]], axis=1))
        wv_c = np.ascontiguousarray(Wqkv[:, 2048 + 512 * g:2048 + 512 * g + 512])
        wout_c = np.ascontiguousarray(Wout[512 * g:512 * g + 512, :])
        in_maps.append({"xT": xTc, "wqk": wqk_c, "wv": wv_c, "wout": wout_c,
                        "cosb": cosb, "sinb": sinb})
    return in_maps


def gather_output(results, B=4):
    outs = []
    for b in range(B):
        acc = results[2 * b]["outT"].astype(np.float32) + results[2 * b + 1]["outT"]
        outs.append(acc.T)
    return np.stack(outs, axis=0)


def kernel(x, Wqkv, Wout):
    x = np.asarray(x, dtype=np.float32)
    Wqkv = np.asarray(Wqkv, dtype=np.float32)
    Wout = np.asarray(Wout, dtype=np.float32)
    nc = _get_prog()
    in_maps = make_in_maps(x, Wqkv, Wout)
    res = run_bass_kernel_spmd(nc, in_maps, core_ids=list(range(N_CORES)))
    return gather_output(res.results, B=x.shape[0])


if __name__ == "__main__":
    rng = np.random.default_rng(0)
    x = rng.standard_normal((4, S, D)).astype(np.float32)
    Wqkv = (rng.standard_normal((D, 3 * D)) * D ** -0.5).astype(np.float32)
    Wout = (rng.standard_normal((D, D)) * D ** -0.5).astype(np.float32)
    out = kernel(x, Wqkv, Wout)
    print("kernel ran, out shape:", out.shape, "finite:", np.isfinite(out).all())
